# revision 16
# baseline (speedup 1.0000x reference)
"""Trainium2 Bass kernel for nn_DenseGraphConvNodeToEdge.

out[b,i,j,o] = y_cols[b,j,o] + y_rows[b,i,o] + y_sum[b,o] + bias[o]
  with y_cols = x @ W0.T, y_rows = x @ W1.T, y_sum = x.sum(1) @ W2.T

Strategy: output is [4,1024,1024,64] f32 = 1 GiB; the problem is pure
memory-regime (tiny GEMMs, huge broadcast-add materialization). Shard the
row dim i across 8 cores (128 rows/core -> 128 MiB of output writes each).

Per core, every [128 i x 512 (j,o)] output tile is produced by TWO fp32r
matmuls accumulating in one PSUM bank (fp32r streams 1 cyc/col vs fp32's
8 cyc/col; fp32r keeps 12 mantissa bits, so a hi+lo split is fp32-exact):

    mm1 (K=128): [x_r ; x_lo].T @ [W1rep_r ; W1rep_r]   = x @ W1rep_r
    mm2 (K=66):  [x_r ; 1 ; 1].T @ [W1rep_lo ; base_r ; base_lo]
               = x_r @ W1rep_lo + base
    (dropped term x_lo @ W1rep_lo ~ 2^-24 relative)

where base[b,j,o] = y_cols + y_sum + bias is precomputed on-chip by small
exact-fp32 GEMMs, split hi/lo, and flattened into rows 64/65 of the rhs
buffer via SBUF->SBUF DMA. PSUM tiles are copied to SBUF staging (DVE/ACT
alternating) and DMA'd out as 2 MiB transfers.
Roofline: 128 MiB / ~360 GB/s ~= 375 us per core.
"""

import numpy as np

B, N, C = 4, 1024, 64
N_CORES = 8
R = N // N_CORES  # 128 rows per core

_CACHE = {}


def _build():
    import concourse.tile as tile
    from concourse import bacc, mybir

    f32 = mybir.dt.float32
    f32r = mybir.dt.float32r

    nc = bacc.Bacc("TRN2", target_bir_lowering=False, debug=False,
                   num_devices=N_CORES)

    xt1 = nc.dram_tensor("xt1", [B, C + 1, N], f32, kind="ExternalInput").ap()
    xrt1 = nc.dram_tensor("xrt1", [B, C + 1, R], f32, kind="ExternalInput").ap()
    w1rep = nc.dram_tensor("w1rep", [C, 512], f32, kind="ExternalInput").ap()
    w0t = nc.dram_tensor("w0t", [C, C], f32, kind="ExternalInput").ap()
    w2t = nc.dram_tensor("w2t", [C, C], f32, kind="ExternalInput").ap()
    bias_row = nc.dram_tensor("bias_row", [1, C], f32, kind="ExternalInput").ap()
    out_s = nc.dram_tensor("out_s", [B, R, N, C], f32, kind="ExternalOutput").ap()

    with tile.TileContext(nc) as tc:
        with (
            tc.tile_pool(name="const", bufs=1) as const_pool,
            tc.tile_pool(name="rhs", bufs=1) as rhs_pool,
            tc.tile_pool(name="base", bufs=16) as base_pool,
            tc.tile_pool(name="stage", bufs=3) as stage_pool,
            tc.tile_pool(name="psm", bufs=3, space="PSUM") as psum_main,
            tc.tile_pool(name="pss", bufs=2, space="PSUM") as psum_small,
        ):
            # ---- persistent SBUF state ----
            xt1_sb = const_pool.tile([C + 1, B * N], f32, tag="xt1")
            xrt1_sb = const_pool.tile([C + 1, B * R], f32, tag="xrt1")
            rhs_base = const_pool.tile([C + 1, C], f32, tag="rhsb")
            w2t_sb = const_pool.tile([C, C], f32, tag="w2t")
            bias_sb = const_pool.tile([1, C], f32, tag="bias")
            xsum_sb = const_pool.tile([C, 1], f32, tag="xsum")
            w1t_tmp = const_pool.tile([C, 512], f32, tag="w1t")
            w1lo_t = const_pool.tile([C, 512], f32r, tag="w1lo")
            xlo_tmp = const_pool.tile([C, R], f32r, tag="xlo")
            # fp32r operands for the main GEMM
            lhsT1_sb = const_pool.tile([128, B * R], f32r, tag="lhsT1")
            lhsT2_sb = const_pool.tile([C + 2, B * R], f32r, tag="lhsT2")
            rhs1_sb = const_pool.tile([128, 512], f32r, tag="rhs1")
            rhs2_bufs = [rhs_pool.tile([C + 2, 8192], f32r, tag=f"rhs2{k}",
                                       name=f"rhs2{k}")
                         for k in range(3)]

            # ---- input DMAs ----
            for b in range(B):
                nc.sync.dma_start(xt1_sb[:, b * N:(b + 1) * N], xt1[b])
                nc.sync.dma_start(xrt1_sb[:, b * R:(b + 1) * R], xrt1[b])
            nc.sync.dma_start(rhs_base[0:C, :], w0t[:, :])
            nc.sync.dma_start(w2t_sb[:], w2t[:, :])
            nc.sync.dma_start(bias_sb[:], bias_row[:, :])
            nc.sync.dma_start(w1t_tmp[:], w1rep[:, :])

            # ---- W1rep hi/lo split ----
            # rhs1 rows 0-63 = round_fp32r(W1rep); rows 64-127 = same bits
            nc.vector.tensor_copy(rhs1_sb[0:C, :], w1t_tmp[:])
            nc.vector.tensor_sub(w1lo_t[:], w1t_tmp[:],
                                 rhs1_sb[0:C, :].bitcast(f32))
            nc.sync.dma_start(rhs1_sb[C:2 * C, :], rhs1_sb[0:C, :])
            # rhs2 rows 0-63 = W1rep_lo tiled 16x along free dim
            # (gpsimd/SWDGE: keep the sync HWDGE FIFO free for output DMAs).
            # Only buffer 0 is filled up front; buffers 1/2 are filled right
            # after the first flattens so chunk 0 isn't stuck behind 48 fills.
            def fill_rhs2(k):
                src = (w1lo_t[:].rearrange("p (one f) -> p one f", one=1)
                       .to_broadcast([C, 16, 512]))
                nc.gpsimd.dma_start(
                    rhs2_bufs[k][0:C, :].rearrange("p (rep f) -> p rep f",
                                                   rep=16),
                    src)

            fill_rhs2(0)

            # ---- x hi/lo split (per b) ----
            # ones rows for lhsT2: round-copy the ones row of xrt1 (fp32r
            # memset is rejected by codegen), then byte-copy row 64 -> 65
            nc.vector.tensor_copy(lhsT2_sb[C:C + 1, :], xrt1_sb[C:C + 1, :])
            nc.sync.dma_start(lhsT2_sb[C + 1:C + 2, :], lhsT2_sb[C:C + 1, :])
            for b in range(B):
                bc = slice(b * R, (b + 1) * R)
                nc.vector.tensor_copy(lhsT1_sb[0:C, bc], xrt1_sb[0:C, bc])
                nc.vector.tensor_sub(xlo_tmp[:], xrt1_sb[0:C, bc],
                                     lhsT1_sb[0:C, bc].bitcast(f32))
                nc.sync.dma_start(lhsT1_sb[C:2 * C, bc], xlo_tmp[:])
                nc.vector.tensor_copy(lhsT2_sb[0:C, bc], xrt1_sb[0:C, bc])

            copy_idx = 0  # alternate DVE / ACT for PSUM->SBUF copies
            for b in range(B):
                # xsum[c] = sum_j x[b,j,c]
                nc.vector.reduce_sum(
                    xsum_sb[:], xt1_sb[0:C, b * N:(b + 1) * N],
                    axis=mybir.AxisListType.X)
                # s2_row[o] = sum_c xsum[c] * W2[o,c] + bias[o]
                ps_s2 = psum_small.tile([1, C], f32, tag="pss")
                nc.tensor.matmul(ps_s2[:], xsum_sb[:], w2t_sb[:],
                                 start=True, stop=True)
                nc.vector.tensor_add(rhs_base[C:C + 1, :], ps_s2[:], bias_sb[:])

                # precompute all 8 base hi/lo tile pairs for this b up front
                # so the per-chunk critical chain is only flatten-DMA -> mm
                base_tiles = []
                for jblk in range(8):
                    # base tile [128 j, 64 o] (exact fp32 GEMM)
                    ps_b = psum_small.tile([128, C], f32, tag="pss")
                    nc.tensor.matmul(
                        ps_b[:],
                        xt1_sb[:, b * N + jblk * 128: b * N + (jblk + 1) * 128],
                        rhs_base[:],
                        start=True, stop=True)
                    base_r = base_pool.tile([128, C], f32r, tag="base",
                                            name=f"base_r_{b}_{jblk}")
                    base_lo = base_pool.tile([128, C], f32r, tag="base",
                                             name=f"base_lo_{b}_{jblk}")
                    nc.vector.tensor_copy(base_r[:], ps_b[:])
                    nc.vector.tensor_sub(base_lo[:], ps_b[:],
                                         base_r[:].bitcast(f32))
                    base_tiles.append((base_r, base_lo))

                lhsT1 = lhsT1_sb[:, b * R:(b + 1) * R]
                lhsT2 = lhsT2_sb[:, b * R:(b + 1) * R]
                for jblk in range(8):
                    base_r, base_lo = base_tiles[jblk]
                    # flatten [128 j, 64 o] -> rows 64/65 of the rhs2 buffer
                    # (gpsimd/SWDGE: don't queue behind 2 MiB output DMAs on
                    # the sync HWDGE FIFO — the matmuls below block on these)
                    rhs2 = rhs2_bufs[(b * 8 + jblk) % 3]
                    nc.gpsimd.dma_start(
                        rhs2[C:C + 1, :].rearrange("a (p o) -> a p o", p=128),
                        base_r[:])
                    nc.gpsimd.dma_start(
                        rhs2[C + 1:C + 2, :].rearrange("a (p o) -> a p o", p=128),
                        base_lo[:])
                    if b == 0 and jblk < 2:
                        fill_rhs2(jblk + 1)

                    # main GEMMs: 16 x [128, 512] tiles = [128 i, 128 j x 64 o]
                    # issue order mm1,mm1,mm2,mm2 per psum tile so each
                    # stationary (lhsT1 / lhsT2) is loaded once per 2 tiles
                    for half in range(2):  # two staging tiles of 4096
                        stage_t = stage_pool.tile([128, 4096], f32, tag="stage")
                        for g in range(4):  # psum groups of [128, 1024]
                            ps_m = psum_main.tile([128, 1024], f32, tag="psm")
                            for h in range(2):
                                nc.tensor.matmul(
                                    ps_m[:, h * 512:(h + 1) * 512],
                                    lhsT1, rhs1_sb[:],
                                    start=True, stop=False)
                            for h in range(2):
                                t = half * 8 + g * 2 + h
                                nc.tensor.matmul(
                                    ps_m[:, h * 512:(h + 1) * 512],
                                    lhsT2, rhs2[:, t * 512:(t + 1) * 512],
                                    start=False, stop=True)
                            dst = stage_t[:, g * 1024:(g + 1) * 1024]
                            if copy_idx % 2 == 0:
                                nc.vector.tensor_copy(dst, ps_m[:])
                            else:
                                nc.scalar.copy(dst, ps_m[:])
                            copy_idx += 1
                        j0 = jblk * 128 + half * 64
                        # alternate sync/scalar HWDGE rings so consecutive
                        # 2 MiB writes overlap their completion latency
                        dma_eng = nc.sync if (copy_idx // 4) % 2 == 0 else nc.scalar
                        dma_eng.dma_start(out_s[b, :, j0:j0 + 64, :], stage_t[:])

    nc.compile()
    return nc


def _get_nc():
    if "nc" not in _CACHE:
        _CACHE["nc"] = _build()
    return _CACHE["nc"]


def kernel(x, adj, W0, W1, W2, bias):
    from concourse.bass_utils import run_bass_kernel_spmd

    x = np.ascontiguousarray(np.asarray(x, dtype=np.float32))
    W0 = np.asarray(W0, dtype=np.float32)
    W1 = np.asarray(W1, dtype=np.float32)
    W2 = np.asarray(W2, dtype=np.float32)
    bias = np.asarray(bias, dtype=np.float32)

    nc = _get_nc()

    ones_n = np.ones((B, 1, N), dtype=np.float32)
    xt1 = np.ascontiguousarray(
        np.concatenate([x.transpose(0, 2, 1), ones_n], axis=1))
    w1rep = np.ascontiguousarray(np.tile(W1.T, (1, 8)))
    w0t = np.ascontiguousarray(W0.T)
    w2t = np.ascontiguousarray(W2.T)
    bias_row = np.ascontiguousarray(bias.T)

    in_maps = []
    ones_r = np.ones((B, 1, R), dtype=np.float32)
    for c in range(N_CORES):
        xr = x[:, c * R:(c + 1) * R, :]
        xrt1 = np.ascontiguousarray(
            np.concatenate([xr.transpose(0, 2, 1), ones_r], axis=1))
        in_maps.append({
            "xt1": xt1, "xrt1": xrt1, "w1rep": w1rep,
            "w0t": w0t, "w2t": w2t, "bias_row": bias_row,
        })

    global _last_in_maps
    _last_in_maps = in_maps
    res = run_bass_kernel_spmd(nc, in_maps, list(range(N_CORES)))

    out = np.empty((B, N, N, C), dtype=np.float32)
    for c in range(N_CORES):
        out[:, c * R:(c + 1) * R] = res.results[c]["out_s"]
    return out
